# revision 8
# baseline (speedup 1.0000x reference)
"""Trainium2 Bass kernel for LGeM self-attention (b=2, t=2048, c=2048, h=16, d=128).

Sharding: 8 cores = 2 (batch, data-parallel) x 4 (head-groups of 4 heads,
tensor-parallel 'mp'). Each core computes q/k/v projections for its 4 heads,
attention, and a partial output projection (its 512 rows of Wo); the host
sums the 4 mp-partials per batch.

Math notes (matching the reference exactly):
  - rope here is q*(cos+sin) elementwise (the module's rotate_half is identity),
    folded with the 1/sqrt(t) logit scale into a precomputed per-(d,t) factor.
  - softmax is computed without max-subtraction: logits are ~N(0, 0.2^2) so
    exp never overflows; exp(x)/sum(exp(x)) == softmax(x) exactly in real math.
  - all matmul operands are bf16 (fp32 PSUM accumulation). rel err ~4e-3 vs
    the f32 reference (tolerance 2e-2).

Performance notes:
  - everything is SBUF-resident: no DRAM scratch roundtrips.
  - the host pre-permutes x/W into partition-major layout so each tensor loads
    with ONE large contiguous DMA (23 DMAs, ~26 MB total per core, vs 326
    small DMAs / 74 MB for the f32 + DRAM-scratch version). Real-HW DMA is
    completion-latency dominated for small transfers; big transfers stream.
  - scores are built transposed, S_T[tk, tq] = k_T_tile.T @ q_T, so attn@v
    needs no transposes (stationary v[tk,d], moving exp(S_T)), and the softmax
    denominator comes from a ones[128,128] stationary matmul which also
    replicates the sums across all partitions.
  - the attention inner loop is software-pipelined: accumulation matmuls for
    key-tile tk are emitted after the scores matmul of tk+1, so PE has
    independent work while ACT computes exp.
  - partial outputs are written bf16 (halves output traffic); host sums in f32.
"""

import sys

sys.path.insert(0, "/opt/trn_rl_repo")

import math

import numpy as np
import ml_dtypes

import concourse.bass as bass
import concourse.mybir as mybir
import concourse.tile as tile
from concourse import bacc, bass_utils

F32 = mybir.dt.float32
BF16 = mybir.dt.bfloat16
BF_NP = ml_dtypes.bfloat16

HIDDEN = 2048
HEADS = 16
HEAD_DIM = 128
SEQ = 2048
BATCH = 2
N_CORES = 8
MP = 4  # tensor-parallel cores per batch
HG = HEADS // MP  # heads per core
THETA = 10000.0


def build_attention_nc(T, C, HG, D, use_mask=False):
    """Build the per-core Bass program. DG = HG*D output dims per core.

    DRAM inputs are host-prepermuted to partition-major [128, chunks*width]:
      xT_p[p, cc*T + t]   = x.T[cc*128 + p, t]
      wq_p[p, cc*DG + j]  = Wq[cc*128 + p, g*DG + j]      (wk, wv likewise)
      wo_p[p, h*C + j]    = Wo[g*DG + h*128 + p, j]
    so each loads with a single contiguous DMA.
    """
    DG = HG * D
    CCH = C // 128  # contraction chunks for projections
    TQC = min(512, T)  # moving-dim chunk (tq)
    NTQ = T // TQC
    NTK = T // 128  # key tiles
    NQT = T // 128  # query row tiles (out proj)
    NOC = C // TQC  # out-proj column chunks

    nc = bacc.Bacc("TRN2", target_bir_lowering=False, debug=False)

    xT = nc.dram_tensor("xT", [128, CCH * T], BF16, kind="ExternalInput").ap()
    wq = nc.dram_tensor("wq", [128, CCH * DG], BF16, kind="ExternalInput").ap()
    wk = nc.dram_tensor("wk", [128, CCH * DG], BF16, kind="ExternalInput").ap()
    wv = nc.dram_tensor("wv", [128, CCH * DG], BF16, kind="ExternalInput").ap()
    wo = nc.dram_tensor("wo", [128, HG * C], BF16, kind="ExternalInput").ap()
    cfq = nc.dram_tensor("cfq", [D, T], F32, kind="ExternalInput").ap()
    cfk = nc.dram_tensor("cfk", [D, T], F32, kind="ExternalInput").ap()
    if use_mask:
        maskT = nc.dram_tensor("maskT", [T, T], F32, kind="ExternalInput").ap()
    out = nc.dram_tensor("out", [T, C], BF16, kind="ExternalOutput").ap()

    with tile.TileContext(nc) as tc:
        # Live across all phases: per-head q/k (as [d, t]), v, attention output.
        with tc.tile_pool(name="persist", bufs=1) as pp:
            qT_sb = pp.tile([128, HG * T], BF16, tag="qT")
            kT_sb = pp.tile([128, HG * T], BF16, tag="kT")
            v_sb = pp.tile([128, NTK * DG], BF16, tag="v")
            out2_sb = pp.tile([128, HG * T], BF16, tag="out2")

            # ---------------- Phase A: projections ----------------
            with (
                tc.tile_pool(name="xp", bufs=1) as xpool,
                tc.tile_pool(name="wp", bufs=1) as wpool,
                tc.tile_pool(name="cfp", bufs=1) as cfpool,
                tc.tile_pool(name="vps", bufs=3, space="PSUM") as vps,
                tc.tile_pool(name="qkps", bufs=4, space="PSUM") as qkps,
            ):
                xT_sb = xpool.tile([128, CCH * T], BF16)
                wq_sb = wpool.tile([128, CCH * DG], BF16, tag="wq")
                wk_sb = wpool.tile([128, CCH * DG], BF16, tag="wk")
                wv_sb = wpool.tile([128, CCH * DG], BF16, tag="wv")
                cfq_sb = cfpool.tile([128, T], F32, tag="cfq")
                cfk_sb = cfpool.tile([128, T], F32, tag="cfk")
                nc.sync.dma_start(xT_sb[:], xT)
                nc.scalar.dma_start(wv_sb[:], wv)
                nc.scalar.dma_start(wq_sb[:], wq)
                nc.scalar.dma_start(wk_sb[:], wk)
                nc.sync.dma_start(cfq_sb[:D, :], cfq)
                nc.sync.dma_start(cfk_sb[:D, :], cfk)

                # v = x @ Wv  ->  v[tk, dout], stationary xT tiles, moving Wv
                for tk in range(NTK):
                    pv = vps.tile([128, DG], F32)
                    for cc in range(CCH):
                        nc.tensor.matmul(
                            pv[:],
                            xT_sb[:, cc * T + tk * 128 : cc * T + (tk + 1) * 128],
                            wv_sb[:, cc * DG : (cc + 1) * DG],
                            start=(cc == 0),
                            stop=(cc == CCH - 1),
                        )
                    nc.vector.tensor_copy(v_sb[:, tk * DG : (tk + 1) * DG], pv[:])

                # q_T = (Wq_h).T @ x_T (then * cfq), k_T likewise (* cfk)
                for h in range(HG):
                    for w_sb, cf_sb, dst in (
                        (wq_sb, cfq_sb, qT_sb),
                        (wk_sb, cfk_sb, kT_sb),
                    ):
                        for tq in range(NTQ):
                            pm = qkps.tile([128, TQC], F32)
                            for cc in range(CCH):
                                nc.tensor.matmul(
                                    pm[:],
                                    w_sb[:, cc * DG + h * D : cc * DG + (h + 1) * D],
                                    xT_sb[:, cc * T + tq * TQC : cc * T + (tq + 1) * TQC],
                                    start=(cc == 0),
                                    stop=(cc == CCH - 1),
                                )
                            nc.vector.tensor_mul(
                                dst[:D, h * T + tq * TQC : h * T + (tq + 1) * TQC],
                                pm[:D, :],
                                cf_sb[:D, tq * TQC : (tq + 1) * TQC],
                            )

            # ---------------- Phase B: attention ----------------
            with (
                tc.tile_pool(name="cst", bufs=1) as cstpool,
                tc.tile_pool(name="ep", bufs=6) as epool,
                tc.tile_pool(name="mp", bufs=4) as mpool,
                tc.tile_pool(name="rp", bufs=2) as rpool,
                tc.tile_pool(name="scps", bufs=4, space="PSUM") as scps,
                tc.tile_pool(name="o2ps", bufs=2, space="PSUM") as o2ps,
                tc.tile_pool(name="sps", bufs=2, space="PSUM") as sps,
            ):
                ones_f = cstpool.tile([128, 128], F32)
                nc.vector.memset(ones_f[:], 1.0)
                ones = cstpool.tile([128, 128], BF16)
                nc.vector.tensor_copy(ones[:], ones_f[:])
                for h in range(HG):
                    for tq in range(NTQ):
                        o2p = o2ps.tile([128, TQC], F32)
                        sp = sps.tile([128, TQC], F32)
                        # Software-pipelined: the accumulation matmuls for
                        # tile tk are emitted after the scores matmul of
                        # tk+1, so PE has independent work while ACT exps.
                        pending = None
                        for tk in range(NTK):
                            scp = scps.tile([128, TQC], F32)
                            nc.tensor.matmul(
                                scp[:],
                                kT_sb[:D, h * T + tk * 128 : h * T + (tk + 1) * 128],
                                qT_sb[:D, h * T + tq * TQC : h * T + (tq + 1) * TQC],
                                start=True,
                                stop=True,
                            )
                            et = epool.tile([128, TQC], BF16, tag="et")
                            if use_mask:
                                mt = mpool.tile([128, TQC], F32, tag="mt")
                                nc.sync.dma_start(
                                    mt[:],
                                    maskT[
                                        tk * 128 : (tk + 1) * 128,
                                        tq * TQC : (tq + 1) * TQC,
                                    ],
                                )
                                ma = mpool.tile([128, TQC], F32, tag="ma")
                                nc.vector.tensor_add(ma[:], scp[:], mt[:])
                                nc.scalar.activation(
                                    et[:],
                                    ma[:],
                                    mybir.ActivationFunctionType.Exp,
                                )
                            else:
                                nc.scalar.activation(
                                    et[:],
                                    scp[:],
                                    mybir.ActivationFunctionType.Exp,
                                )
                            if pending is not None:
                                p_et, p_tk = pending
                                nc.tensor.matmul(
                                    o2p[:],
                                    v_sb[:, p_tk * DG + h * D : p_tk * DG + (h + 1) * D],
                                    p_et[:],
                                    start=(p_tk == 0),
                                    stop=False,
                                )
                                nc.tensor.matmul(
                                    sp[:],
                                    ones[:],
                                    p_et[:],
                                    start=(p_tk == 0),
                                    stop=False,
                                )
                            pending = (et, tk)
                        p_et, p_tk = pending
                        nc.tensor.matmul(
                            o2p[:],
                            v_sb[:, p_tk * DG + h * D : p_tk * DG + (h + 1) * D],
                            p_et[:],
                            start=False,
                            stop=True,
                        )
                        nc.tensor.matmul(
                            sp[:],
                            ones[:],
                            p_et[:],
                            start=False,
                            stop=True,
                        )
                        rt = rpool.tile([128, TQC], F32)
                        nc.vector.reciprocal(rt[:], sp[:])
                        nc.vector.tensor_mul(
                            out2_sb[:D, h * T + tq * TQC : h * T + (tq + 1) * TQC],
                            o2p[:D, :],
                            rt[:D, :],
                        )

            # -------- output projection (partial over this core's heads) ----
            with (
                tc.tile_pool(name="wop", bufs=1) as wopool,
                tc.tile_pool(name="fst", bufs=3) as fpool,
                tc.tile_pool(name="fps", bufs=4, space="PSUM") as fps,
            ):
                wo_sb = wopool.tile([128, HG * C], BF16)
                nc.scalar.dma_start(wo_sb[:], wo)
                for qt in range(NQT):
                    ft = fpool.tile([128, C], BF16, tag="ft")
                    for oc in range(NOC):
                        fp = fps.tile([128, TQC], F32)
                        for h in range(HG):
                            nc.tensor.matmul(
                                fp[:],
                                out2_sb[:D, h * T + qt * 128 : h * T + (qt + 1) * 128],
                                wo_sb[:D, h * C + oc * TQC : h * C + (oc + 1) * TQC],
                                start=(h == 0),
                                stop=(h == HG - 1),
                            )
                        nc.vector.tensor_copy(
                            ft[:, oc * TQC : (oc + 1) * TQC], fp[:]
                        )
                    eng = nc.sync if qt % 2 == 0 else nc.scalar
                    eng.dma_start(out[qt * 128 : (qt + 1) * 128, :], ft[:])

    nc.compile()
    return nc


def compute_cfacs(T, D, theta=THETA):
    """cfq = (cos+sin).T / sqrt(T)  [D, T];  cfk = (cos+sin).T  [D, T]."""
    freq = 1.0 / theta ** (np.arange(0, D, 2, dtype=np.float64) / D)
    t = np.arange(T, dtype=np.float64)
    m = np.einsum("i,j->ij", t, freq)  # [T, D/2]
    m = np.concatenate([m, m], axis=-1)  # [T, D]
    cfac = (np.cos(m) + np.sin(m)).astype(np.float32)  # [T, D]
    cfk = np.ascontiguousarray(cfac.T)  # [D, T]
    cfq = np.ascontiguousarray(cfac.T / np.float32(math.sqrt(T))).astype(np.float32)
    return cfq, cfk


def _perm_cols(a, chunk=128):
    """[R, W] -> [128, (R//128)*W] with out[p, cc*W + j] = a[cc*128 + p, j]."""
    R, W = a.shape
    n = R // chunk
    return np.ascontiguousarray(
        a.reshape(n, chunk, W).transpose(1, 0, 2).reshape(chunk, n * W)
    )


_NC_CACHE = {}


def _get_nc(use_mask):
    key = bool(use_mask)
    if key not in _NC_CACHE:
        _NC_CACHE[key] = build_attention_nc(SEQ, HIDDEN, HG, HEAD_DIM, use_mask=key)
    return _NC_CACHE[key]


def kernel(input_ids, attention_mask, Wq, Wk, Wv, Wo):
    input_ids = np.asarray(input_ids, dtype=np.float32)
    attention_mask = np.asarray(attention_mask, dtype=np.float32)
    Wq = np.asarray(Wq, dtype=np.float32)
    Wk = np.asarray(Wk, dtype=np.float32)
    Wv = np.asarray(Wv, dtype=np.float32)
    Wo = np.asarray(Wo, dtype=np.float32)

    b, t, c = input_ids.shape
    assert (b, t, c) == (BATCH, SEQ, HIDDEN)
    DG = HG * HEAD_DIM

    use_mask = bool(np.any(attention_mask))
    nc = _get_nc(use_mask)

    cfq, cfk = compute_cfacs(SEQ, HEAD_DIM)

    xT_p = [
        _perm_cols(np.ascontiguousarray(input_ids[bi].T)).astype(BF_NP)
        for bi in range(BATCH)
    ]
    wq_p = [_perm_cols(Wq[:, g * DG : (g + 1) * DG]).astype(BF_NP) for g in range(MP)]
    wk_p = [_perm_cols(Wk[:, g * DG : (g + 1) * DG]).astype(BF_NP) for g in range(MP)]
    wv_p = [_perm_cols(Wv[:, g * DG : (g + 1) * DG]).astype(BF_NP) for g in range(MP)]
    wo_p = [_perm_cols(Wo[g * DG : (g + 1) * DG, :]).astype(BF_NP) for g in range(MP)]

    in_maps = []
    for core in range(N_CORES):
        bi, g = divmod(core, MP)
        m = {
            "xT": xT_p[bi],
            "wq": wq_p[g],
            "wk": wk_p[g],
            "wv": wv_p[g],
            "wo": wo_p[g],
            "cfq": cfq,
            "cfk": cfk,
        }
        if use_mask:
            m["maskT"] = np.ascontiguousarray(attention_mask[bi, 0].T)
        in_maps.append(m)

    res = bass_utils.run_bass_kernel_spmd(nc, in_maps, core_ids=list(range(N_CORES)))

    out = np.zeros((BATCH, SEQ, HIDDEN), dtype=np.float32)
    for bi in range(BATCH):
        acc = res.results[bi * MP]["out"].astype(np.float32)
        for g in range(1, MP):
            acc = acc + res.results[bi * MP + g]["out"].astype(np.float32)
        out[bi] = acc
    return out


# revision 10
# speedup vs baseline: 1.0534x; 1.0534x over previous
"""Trainium2 Bass kernel for LGeM self-attention (b=2, t=2048, c=2048, h=16, d=128).

Sharding: 8 cores = 2 (batch, data-parallel) x 4 (head-groups of 4 heads,
tensor-parallel 'mp'). Each core computes q/k/v projections for its 4 heads,
attention, and a partial output projection (its 512 rows of Wo); the host
sums the 4 mp-partials per batch.

Math notes (matching the reference exactly):
  - rope here is q*(cos+sin) elementwise (the module's rotate_half is identity),
    folded with the 1/sqrt(t) logit scale into a precomputed per-(d,t) factor.
  - softmax is computed without max-subtraction: logits are ~N(0, 0.2^2) so
    exp never overflows; exp(x)/sum(exp(x)) == softmax(x) exactly in real math.
  - all matmul operands are bf16 (fp32 PSUM accumulation). rel err ~4e-3 vs
    the f32 reference (tolerance 2e-2).

Performance notes:
  - everything is SBUF-resident: no DRAM scratch roundtrips.
  - the host pre-permutes x/W into partition-major layout so each tensor loads
    with ONE large contiguous DMA (23 DMAs, ~26 MB total per core, vs 326
    small DMAs / 74 MB for the f32 + DRAM-scratch version). Real-HW DMA is
    completion-latency dominated for small transfers; big transfers stream.
  - scores are built transposed, S_T[tk, tq] = k_T_tile.T @ q_T, so attn@v
    needs no transposes (stationary v[tk,d], moving exp(S_T)), and the softmax
    denominator comes from a ones[128,128] stationary matmul which also
    replicates the sums across all partitions.
  - the attention inner loop is software-pipelined: accumulation matmuls for
    key-tile tk are emitted after the scores matmul of tk+1, so PE has
    independent work while ACT computes exp.
  - partial outputs are written bf16 (halves output traffic); host sums in f32.
"""

import sys

sys.path.insert(0, "/opt/trn_rl_repo")

import math

import numpy as np
import ml_dtypes

import concourse.bass as bass
import concourse.mybir as mybir
import concourse.tile as tile
from concourse import bacc, bass_utils

F32 = mybir.dt.float32
BF16 = mybir.dt.bfloat16
BF_NP = ml_dtypes.bfloat16

HIDDEN = 2048
HEADS = 16
HEAD_DIM = 128
SEQ = 2048
BATCH = 2
N_CORES = 8
MP = 4  # tensor-parallel cores per batch
HG = HEADS // MP  # heads per core
THETA = 10000.0


def build_attention_nc(T, C, HG, D, use_mask=False):
    """Build the per-core Bass program. DG = HG*D output dims per core.

    DRAM inputs are host-prepermuted to partition-major [128, chunks*width]:
      xT_p[p, cc*T + t]   = x.T[cc*128 + p, t]
      wq_p[p, cc*DG + j]  = Wq[cc*128 + p, g*DG + j]      (wk, wv likewise)
      wo_p[p, h*C + j]    = Wo[g*DG + h*128 + p, j]
    so each loads with a single contiguous DMA.
    """
    DG = HG * D
    CCH = C // 128  # contraction chunks for projections
    TQC = min(512, T)  # moving-dim chunk (tq)
    NTQ = T // TQC
    NTK = T // 128  # key tiles
    NQT = T // 128  # query row tiles (out proj)
    NOC = C // TQC  # out-proj column chunks

    nc = bacc.Bacc("TRN2", target_bir_lowering=False, debug=False)

    xT = nc.dram_tensor("xT", [128, CCH * T], BF16, kind="ExternalInput").ap()
    wq = nc.dram_tensor("wq", [128, CCH * DG], BF16, kind="ExternalInput").ap()
    wk = nc.dram_tensor("wk", [128, CCH * DG], BF16, kind="ExternalInput").ap()
    wv = nc.dram_tensor("wv", [128, CCH * DG], BF16, kind="ExternalInput").ap()
    wo = nc.dram_tensor("wo", [128, HG * C], BF16, kind="ExternalInput").ap()
    cfq = nc.dram_tensor("cfq", [D, T], F32, kind="ExternalInput").ap()
    cfk = nc.dram_tensor("cfk", [D, T], F32, kind="ExternalInput").ap()
    if use_mask:
        maskT = nc.dram_tensor("maskT", [T, T], F32, kind="ExternalInput").ap()
    out = nc.dram_tensor("out", [T, C], BF16, kind="ExternalOutput").ap()

    with tile.TileContext(nc) as tc:
        # Live across all phases: per-head q/k (as [d, t]), v, attention output.
        with tc.tile_pool(name="persist", bufs=1) as pp:
            qT_sb = pp.tile([128, HG * T], BF16, tag="qT")
            kT_sb = pp.tile([128, HG * T], BF16, tag="kT")
            v_sb = pp.tile([128, NTK * DG], BF16, tag="v")
            out2_sb = pp.tile([128, HG * T], BF16, tag="out2")

            # ------- Phases A+B: projections software-pipelined with attention
            # The q/k projection matmuls for head h+1 are interleaved into
            # head h's attention tk-loop as PE "filler": whenever PE would
            # stall waiting for ACT's exp, it has always-ready projection
            # work instead, and ACT (idle during pure projection phases)
            # overlaps with projection compute.
            with (
                tc.tile_pool(name="xp", bufs=1) as xpool,
                tc.tile_pool(name="wp", bufs=1) as wpool,
                tc.tile_pool(name="cfp", bufs=1) as cfpool,
                tc.tile_pool(name="cst", bufs=1) as cstpool,
                tc.tile_pool(name="ep", bufs=6) as epool,
                tc.tile_pool(name="mp", bufs=4) as mpool,
                tc.tile_pool(name="rp", bufs=2) as rpool,
                tc.tile_pool(name="projps", bufs=2, space="PSUM") as projps,
                tc.tile_pool(name="scps", bufs=4, space="PSUM") as scps,
                tc.tile_pool(name="o2ps", bufs=1, space="PSUM") as o2ps,
                tc.tile_pool(name="sps", bufs=1, space="PSUM") as sps,
            ):
                xT_sb = xpool.tile([128, CCH * T], BF16)
                wq_sb = wpool.tile([128, CCH * DG], BF16, tag="wq")
                wk_sb = wpool.tile([128, CCH * DG], BF16, tag="wk")
                wv_sb = wpool.tile([128, CCH * DG], BF16, tag="wv")
                cfq_sb = cfpool.tile([128, T], F32, tag="cfq")
                cfk_sb = cfpool.tile([128, T], F32, tag="cfk")
                nc.sync.dma_start(xT_sb[:], xT)
                nc.scalar.dma_start(wv_sb[:], wv)
                nc.scalar.dma_start(wq_sb[:], wq)
                nc.scalar.dma_start(wk_sb[:], wk)
                nc.sync.dma_start(cfq_sb[:D, :], cfq)
                nc.sync.dma_start(cfk_sb[:D, :], cfk)

                ones_f = cstpool.tile([128, 128], F32)
                nc.vector.memset(ones_f[:], 1.0)
                ones = cstpool.tile([128, 128], BF16)
                nc.vector.tensor_copy(ones[:], ones_f[:])

                def qk_proj_gen(h):
                    """Emit head h's q/k projections one instruction at a
                    time (yield after each), for interleaving."""
                    for w_sb, cf_sb, dst in (
                        (wq_sb, cfq_sb, qT_sb),
                        (wk_sb, cfk_sb, kT_sb),
                    ):
                        for tq in range(NTQ):
                            pm = projps.tile([128, TQC], F32, tag="pm", name="pm")
                            for cc in range(CCH):
                                nc.tensor.matmul(
                                    pm[:],
                                    w_sb[:, cc * DG + h * D : cc * DG + (h + 1) * D],
                                    xT_sb[:, cc * T + tq * TQC : cc * T + (tq + 1) * TQC],
                                    start=(cc == 0),
                                    stop=(cc == CCH - 1),
                                )
                                yield
                            nc.vector.tensor_mul(
                                dst[:D, h * T + tq * TQC : h * T + (tq + 1) * TQC],
                                pm[:D, :],
                                cf_sb[:D, tq * TQC : (tq + 1) * TQC],
                            )
                            yield

                # v = x @ Wv  ->  v[tk, dout], stationary xT tiles, moving Wv
                for tk in range(NTK):
                    pv = projps.tile([128, TQC], F32, tag="pm", name="pv")
                    for cc in range(CCH):
                        nc.tensor.matmul(
                            pv[:],
                            xT_sb[:, cc * T + tk * 128 : cc * T + (tk + 1) * 128],
                            wv_sb[:, cc * DG : (cc + 1) * DG],
                            start=(cc == 0),
                            stop=(cc == CCH - 1),
                        )
                    nc.vector.tensor_copy(v_sb[:, tk * DG : (tk + 1) * DG], pv[:])

                # q/k projections for head 0 (not interleaved)
                for _ in qk_proj_gen(0):
                    pass

                # attention head h + interleaved q/k projections for head h+1
                for h in range(HG):
                    filler = qk_proj_gen(h + 1) if h + 1 < HG else iter(())
                    for tq in range(NTQ):
                        o2p = o2ps.tile([128, TQC], F32)
                        sp = sps.tile([128, TQC], F32)
                        pending = []
                        for tk in range(NTK):
                            scp = scps.tile([128, TQC], F32)
                            nc.tensor.matmul(
                                scp[:],
                                kT_sb[:D, h * T + tk * 128 : h * T + (tk + 1) * 128],
                                qT_sb[:D, h * T + tq * TQC : h * T + (tq + 1) * TQC],
                                start=True,
                                stop=True,
                            )
                            et = epool.tile([128, TQC], BF16, tag="et")
                            if use_mask:
                                mt = mpool.tile([128, TQC], F32, tag="mt")
                                nc.sync.dma_start(
                                    mt[:],
                                    maskT[
                                        tk * 128 : (tk + 1) * 128,
                                        tq * TQC : (tq + 1) * TQC,
                                    ],
                                )
                                ma = mpool.tile([128, TQC], F32, tag="ma")
                                nc.vector.tensor_add(ma[:], scp[:], mt[:])
                                nc.scalar.activation(
                                    et[:],
                                    ma[:],
                                    mybir.ActivationFunctionType.Exp,
                                )
                            else:
                                nc.scalar.activation(
                                    et[:],
                                    scp[:],
                                    mybir.ActivationFunctionType.Exp,
                                )
                            # two filler projection ops per tk step
                            next(filler, None)
                            next(filler, None)
                            pending.append((et, tk))
                            if len(pending) > 2:
                                p_et, p_tk = pending.pop(0)
                                nc.tensor.matmul(
                                    o2p[:],
                                    v_sb[:, p_tk * DG + h * D : p_tk * DG + (h + 1) * D],
                                    p_et[:],
                                    start=(p_tk == 0),
                                    stop=False,
                                )
                                nc.tensor.matmul(
                                    sp[:],
                                    ones[:],
                                    p_et[:],
                                    start=(p_tk == 0),
                                    stop=False,
                                )
                        for p_et, p_tk in pending:
                            nc.tensor.matmul(
                                o2p[:],
                                v_sb[:, p_tk * DG + h * D : p_tk * DG + (h + 1) * D],
                                p_et[:],
                                start=(p_tk == 0),
                                stop=(p_tk == NTK - 1),
                            )
                            nc.tensor.matmul(
                                sp[:],
                                ones[:],
                                p_et[:],
                                start=(p_tk == 0),
                                stop=(p_tk == NTK - 1),
                            )
                        rt = rpool.tile([128, TQC], F32)
                        nc.vector.reciprocal(rt[:], sp[:])
                        nc.vector.tensor_mul(
                            out2_sb[:D, h * T + tq * TQC : h * T + (tq + 1) * TQC],
                            o2p[:D, :],
                            rt[:D, :],
                        )
                    # make sure head h+1's projections are fully emitted
                    for _ in filler:
                        pass

            # -------- output projection (partial over this core's heads) ----
            with (
                tc.tile_pool(name="wop", bufs=1) as wopool,
                tc.tile_pool(name="fst", bufs=3) as fpool,
                tc.tile_pool(name="fps", bufs=4, space="PSUM") as fps,
            ):
                wo_sb = wopool.tile([128, HG * C], BF16)
                nc.scalar.dma_start(wo_sb[:], wo)
                for qt in range(NQT):
                    ft = fpool.tile([128, C], BF16, tag="ft")
                    for oc in range(NOC):
                        fp = fps.tile([128, TQC], F32)
                        for h in range(HG):
                            nc.tensor.matmul(
                                fp[:],
                                out2_sb[:D, h * T + qt * 128 : h * T + (qt + 1) * 128],
                                wo_sb[:D, h * C + oc * TQC : h * C + (oc + 1) * TQC],
                                start=(h == 0),
                                stop=(h == HG - 1),
                            )
                        nc.vector.tensor_copy(
                            ft[:, oc * TQC : (oc + 1) * TQC], fp[:]
                        )
                    eng = nc.sync if qt % 2 == 0 else nc.scalar
                    eng.dma_start(out[qt * 128 : (qt + 1) * 128, :], ft[:])

    nc.compile()
    return nc


def compute_cfacs(T, D, theta=THETA):
    """cfq = (cos+sin).T / sqrt(T)  [D, T];  cfk = (cos+sin).T  [D, T]."""
    freq = 1.0 / theta ** (np.arange(0, D, 2, dtype=np.float64) / D)
    t = np.arange(T, dtype=np.float64)
    m = np.einsum("i,j->ij", t, freq)  # [T, D/2]
    m = np.concatenate([m, m], axis=-1)  # [T, D]
    cfac = (np.cos(m) + np.sin(m)).astype(np.float32)  # [T, D]
    cfk = np.ascontiguousarray(cfac.T)  # [D, T]
    cfq = np.ascontiguousarray(cfac.T / np.float32(math.sqrt(T))).astype(np.float32)
    return cfq, cfk


def _perm_cols(a, chunk=128):
    """[R, W] -> [128, (R//128)*W] with out[p, cc*W + j] = a[cc*128 + p, j]."""
    R, W = a.shape
    n = R // chunk
    return np.ascontiguousarray(
        a.reshape(n, chunk, W).transpose(1, 0, 2).reshape(chunk, n * W)
    )


_NC_CACHE = {}


def _get_nc(use_mask):
    key = bool(use_mask)
    if key not in _NC_CACHE:
        _NC_CACHE[key] = build_attention_nc(SEQ, HIDDEN, HG, HEAD_DIM, use_mask=key)
    return _NC_CACHE[key]


def kernel(input_ids, attention_mask, Wq, Wk, Wv, Wo):
    input_ids = np.asarray(input_ids, dtype=np.float32)
    attention_mask = np.asarray(attention_mask, dtype=np.float32)
    Wq = np.asarray(Wq, dtype=np.float32)
    Wk = np.asarray(Wk, dtype=np.float32)
    Wv = np.asarray(Wv, dtype=np.float32)
    Wo = np.asarray(Wo, dtype=np.float32)

    b, t, c = input_ids.shape
    assert (b, t, c) == (BATCH, SEQ, HIDDEN)
    DG = HG * HEAD_DIM

    use_mask = bool(np.any(attention_mask))
    nc = _get_nc(use_mask)

    cfq, cfk = compute_cfacs(SEQ, HEAD_DIM)

    xT_p = [
        _perm_cols(np.ascontiguousarray(input_ids[bi].T)).astype(BF_NP)
        for bi in range(BATCH)
    ]
    wq_p = [_perm_cols(Wq[:, g * DG : (g + 1) * DG]).astype(BF_NP) for g in range(MP)]
    wk_p = [_perm_cols(Wk[:, g * DG : (g + 1) * DG]).astype(BF_NP) for g in range(MP)]
    wv_p = [_perm_cols(Wv[:, g * DG : (g + 1) * DG]).astype(BF_NP) for g in range(MP)]
    wo_p = [_perm_cols(Wo[g * DG : (g + 1) * DG, :]).astype(BF_NP) for g in range(MP)]

    in_maps = []
    for core in range(N_CORES):
        bi, g = divmod(core, MP)
        m = {
            "xT": xT_p[bi],
            "wq": wq_p[g],
            "wk": wk_p[g],
            "wv": wv_p[g],
            "wo": wo_p[g],
            "cfq": cfq,
            "cfk": cfk,
        }
        if use_mask:
            m["maskT"] = np.ascontiguousarray(attention_mask[bi, 0].T)
        in_maps.append(m)

    res = bass_utils.run_bass_kernel_spmd(nc, in_maps, core_ids=list(range(N_CORES)))

    out = np.zeros((BATCH, SEQ, HIDDEN), dtype=np.float32)
    for bi in range(BATCH):
        acc = res.results[bi * MP]["out"].astype(np.float32)
        for g in range(1, MP):
            acc = acc + res.results[bi * MP + g]["out"].astype(np.float32)
        out[bi] = acc
    return out
